# revision 13
# baseline (speedup 1.0000x reference)
"""Trainium2 Bass kernel for nn_CodecTransformer_88948772700263.

2-layer dense transformer, B=4, S=2048, D=1024, H=8 heads x 128, FF=4096,
sliding-window-16 causal attention with ALiBi bias, RMS norms, SwiGLU FFN.

Sharding: the 16-token sliding window makes the model sequence-local, so the
8192 tokens are split into 8 shards of 1024 tokens (one per core), each with a
32-token left halo (2 layers x window 16). No collectives. Each core runs the
full 2-layer stack on its 1056-token chunk in a feature-major layout; norm
weights / residual scales are folded into the weight matrices on the host.
"""

import math
import sys

sys.path.insert(0, "/opt/trn_rl_repo")

import ml_dtypes
import numpy as np

import concourse.bass as bass
import concourse.mybir as mybir
import concourse.tile as tile
from concourse import bacc
from concourse.bass_utils import run_bass_kernel_spmd

F32 = mybir.dt.float32
F32R = mybir.dt.float32r
BF16 = mybir.dt.bfloat16

B, S, D, H, HD, FF = 4, 2048, 1024, 8, 128, 4096
L = 2
WINDOW = 16
NORM_EPS = 0.01
QK_EPS = 1e-6

HALO = 32
T = 1024 + HALO          # 1056 tokens per core chunk
KC = D // 128            # 8 feature chunks
MC = D // 128            # 8 output chunks for D-dim outputs
FC = FF // 128           # 32 ff chunks
NSUP = 4                 # ff super chunks of 1024
NTS = [(0, 384), (384, 384), (768, 288)]   # token column tiles (all >=256)
QTS = [(tt * 128, 128) for tt in range(8)] + [(1024, 32)]  # query tiles
NEG = -1e30

# When set, emit Sigmoid+mul instead of Silu (CoreSim lacks Silu; identical
# math: silu(x) = x * sigmoid(x)).
SIM_COMPAT = bool(int(__import__("os").environ.get("BASS_SIM_COMPAT", "0")))

_CACHE = {}


def _build_program():
    nc = bacc.Bacc("TRN2", target_bir_lowering=False, debug=False)

    x_t = nc.dram_tensor("x_t", [D, T], F32, kind="ExternalInput")
    wq_d = nc.dram_tensor("wq_t", [L, KC, MC, 128, 128], F32, kind="ExternalInput")
    wk_d = nc.dram_tensor("wk_t", [L, KC, MC, 128, 128], F32, kind="ExternalInput")
    wv_d = nc.dram_tensor("wv_t", [L, KC, MC, 128, 128], F32, kind="ExternalInput")
    wo_d = nc.dram_tensor("wo_t", [L, KC, MC, 128, 128], F32, kind="ExternalInput")
    w1_d = nc.dram_tensor("w1_t", [L, KC, FC, 128, 128], F32, kind="ExternalInput")
    w3_d = nc.dram_tensor("w3_t", [L, KC, FC, 128, 128], F32, kind="ExternalInput")
    w2_d = nc.dram_tensor("w2_t", [L, FC, MC, 128, 128], BF16, kind="ExternalInput")
    qnw_d = nc.dram_tensor("qnw_t", [128, L, KC], F32, kind="ExternalInput")
    knw_d = nc.dram_tensor("knw_t", [128, L, KC], F32, kind="ExternalInput")
    bias_d = nc.dram_tensor("bias_t", [128, H * 2, 144], BF16, kind="ExternalInput")
    ones_d = nc.dram_tensor("ones_t", [128, 128], BF16, kind="ExternalInput")
    ident_d = nc.dram_tensor("ident_t", [128, 128], BF16, kind="ExternalInput")
    out_t = nc.dram_tensor("out_t", [D, T], F32, kind="ExternalOutput")

    mult = mybir.AluOpType.mult
    AF = mybir.ActivationFunctionType

    with tile.TileContext(nc) as tc:
        with tc.tile_pool(name="const", bufs=1) as constp, \
             tc.tile_pool(name="res", bufs=1) as resp, \
             tc.tile_pool(name="big", bufs=1) as bigp, \
             tc.tile_pool(name="wf", bufs=40) as wfp, \
             tc.tile_pool(name="wb", bufs=16) as wbp, \
             tc.tile_pool(name="scr", bufs=3) as scrp, \
             tc.tile_pool(name="stat", bufs=6) as statp:

            bias_sb = constp.tile([128, H * 2, 144], BF16)
            nc.sync.dma_start(bias_sb[:], bias_d[:])
            ones_sb = constp.tile([128, 128], BF16)
            nc.sync.dma_start(ones_sb[:], ones_d[:])
            ident_sb = constp.tile([128, 128], BF16)
            nc.sync.dma_start(ident_sb[:], ident_d[:])
            qnw_sb = constp.tile([128, L, KC], F32)
            nc.sync.dma_start(qnw_sb[:], qnw_d[:])
            knw_sb = constp.tile([128, L, KC], F32)
            nc.sync.dma_start(knw_sb[:], knw_d[:])

            eps_n = constp.tile([128, 1], F32)
            nc.vector.memset(eps_n[:], NORM_EPS)
            eps_q = constp.tile([128, 1], F32)
            nc.vector.memset(eps_q[:], QK_EPS)

            h = resp.tile([128, KC, T], F32)
            nc.sync.dma_start(h[:], x_t[:].rearrange("(c p) t -> p c t", p=128))

            def rms_norm_into(dst, eps_ap):
                """dst[:, c, t] = h[:, c, t] * rsqrt(mean_d h^2 + eps)."""
                with tc.tile_pool(name="pnorm", bufs=3, space="PSUM") as pn:
                    for t0, tn in NTS:
                        ts = slice(t0, t0 + tn)
                        pss = pn.tile([128, tn], F32, tag="pss")
                        for c in range(KC):
                            sq = scrp.tile([128, tn], BF16, tag="sq")
                            nc.scalar.square(sq[:], h[:, c, ts])
                            nc.tensor.matmul(pss[:], ones_sb[:], sq[:],
                                             start=(c == 0), stop=(c == KC - 1))
                        sd = scrp.tile([128, tn], F32, tag="sd")
                        nc.scalar.activation(sd[:], pss[:], AF.Sqrt,
                                             bias=eps_ap[:], scale=1.0 / D)
                        r = scrp.tile([128, tn], F32, tag="rn")
                        nc.vector.reciprocal(r[:], sd[:])
                        for c in range(KC):
                            nc.vector.tensor_mul(dst[:, c, ts], h[:, c, ts], r[:])

            def proj_qk(l, w_dram, nw_sb, dst, eps_ap):
                """dst = rms_norm(xn @ W, nw) in bf16, feature-major."""
                with tc.tile_pool(name="pqk", bufs=3, space="PSUM") as pq_pool:
                    pssq = []
                    for t0, tn in NTS:
                        pssq.append(pq_pool.tile([128, tn], F32, tag=f"ssq{t0}",
                                                 bufs=1, name=f"pssq{t0}"))
                    for m in range(MC):
                        wts = []
                        for c in range(KC):
                            wt = wfp.tile([128, 128], F32R, tag="wf")
                            nc.sync.dma_start(wt[:], w_dram[l, c, m].bitcast(F32R))
                            wts.append(wt)
                        for i, (t0, tn) in enumerate(NTS):
                            ts = slice(t0, t0 + tn)
                            pq = pq_pool.tile([128, tn], F32, tag="pq")
                            for c in range(KC):
                                nc.tensor.matmul(pq[:], wts[c][:], xn[:, c, ts],
                                                 start=(c == 0), stop=(c == KC - 1))
                            nc.vector.tensor_copy(dst[:, m, ts], pq[:])
                            sq = scrp.tile([128, tn], BF16, tag="sq")
                            nc.scalar.square(sq[:], pq[:])
                            nc.tensor.matmul(pssq[i][:], ones_sb[:], sq[:],
                                             start=(m == 0), stop=(m == MC - 1))
                    for i, (t0, tn) in enumerate(NTS):
                        ts = slice(t0, t0 + tn)
                        sd = scrp.tile([128, tn], F32, tag="sd")
                        nc.scalar.activation(sd[:], pssq[i][:], AF.Sqrt,
                                             bias=eps_ap[:], scale=1.0 / D)
                        r = scrp.tile([128, tn], F32, tag="rq")
                        nc.vector.reciprocal(r[:], sd[:])
                        for m in range(MC):
                            nc.vector.scalar_tensor_tensor(
                                dst[:, m, ts], dst[:, m, ts], nw_sb[:, l, m:m + 1], r[:],
                                op0=mult, op1=mult)

            for l in range(L):
                # ---- attention norm (anw folded into wq/wk/wv on host) ----
                xn = bigp.tile([128, KC, T], F32R, tag="xy", name=f"xn_{l}")
                rms_norm_into(xn, eps_n)

                # ---- q/k projections with flat qk-norm ----
                qb = bigp.tile([128, KC, T], BF16, tag="qb", name=f"qb_{l}")
                proj_qk(l, wq_d, qnw_sb, qb, eps_q)
                kb = bigp.tile([128, KC, T], BF16, tag="kb", name=f"kb_{l}")
                proj_qk(l, wk_d, knw_sb, kb, eps_q)

                # ---- v projection (feature-major bf16) ----
                vf = bigp.tile([128, KC, 16 + T], BF16, tag="g", name=f"vf_{l}")
                with tc.tile_pool(name="pv", bufs=3, space="PSUM") as pvp:
                    for m in range(MC):
                        nc.vector.memset(vf[:, m, 0:16], 0.0)
                        wts = []
                        for c in range(KC):
                            wt = wfp.tile([128, 128], F32R, tag="wf")
                            nc.sync.dma_start(wt[:], wv_d[l, c, m].bitcast(F32R))
                            wts.append(wt)
                        for t0, tn in NTS:
                            ts = slice(t0, t0 + tn)
                            pv = pvp.tile([128, tn], F32, tag="pv")
                            for c in range(KC):
                                nc.tensor.matmul(pv[:], wts[c][:], xn[:, c, ts],
                                                 start=(c == 0), stop=(c == KC - 1))
                            nc.vector.tensor_copy(vf[:, m, 16 + t0:16 + t0 + tn],
                                                  pv[:])

                # ---- transpose v to shifted token-major tiles ----
                # vts tile tt row r = v at token tt*128 - 16 + r (vf is
                # left-padded with 16 zero columns so the shift never
                # underruns; tile 8 only has 48 valid rows).
                vt = bigp.tile([128, 9, D], BF16, tag="vt", name=f"vt_{l}")
                with tc.tile_pool(name="ptr", bufs=4, space="PSUM") as ptp:
                    for tt in range(9):
                        rows = 128 if tt < 8 else 48
                        for c in range(KC):
                            pt = ptp.tile([128, 128], BF16, tag="pt")
                            nc.tensor.transpose(
                                pt[:rows, :], vf[:, c, tt * 128:tt * 128 + rows],
                                ident_sb[:])
                            nc.vector.tensor_copy(
                                vt[:rows, tt, c * 128:(c + 1) * 128], pt[:rows, :])

                # ---- attention ----
                y = bigp.tile([128, KC, T], F32R, tag="xy", name=f"y_{l}")
                with tc.tile_pool(name="pat", bufs=2, space="PSUM") as pat, \
                     tc.tile_pool(name="pat2", bufs=2, space="PSUM") as pat2:
                    for tt, (t0, tq) in enumerate(QTS):
                        nk = 16 + tq
                        slot = 0 if tt == 0 else 1
                        for hh in range(H):
                            bsl = bias_sb[:tq, hh * 2 + slot, :]
                            ps = pat.tile([128, 144], F32, tag="ps")
                            sc = scrp.tile([128, 144], F32, tag="sc")
                            if tt == 0:
                                nc.tensor.matmul(
                                    ps[:tq, 16:nk], qb[:, hh, t0:t0 + tq],
                                    kb[:, hh, 0:tq], start=True, stop=True)
                                nc.vector.tensor_copy(sc[:tq, 0:16], bsl[:, 0:16])
                                nc.vector.tensor_add(
                                    sc[:tq, 16:nk], ps[:tq, 16:nk], bsl[:, 16:nk])
                            else:
                                nc.tensor.matmul(
                                    ps[:tq, 0:nk], qb[:, hh, t0:t0 + tq],
                                    kb[:, hh, t0 - 16:t0 + tq],
                                    start=True, stop=True)
                                nc.vector.tensor_add(
                                    sc[:tq, 0:nk], ps[:tq, 0:nk], bsl[:, 0:nk])
                            nmax = statp.tile([128, 1], F32, tag="nmax")
                            nc.vector.reduce_max(nmax[:tq], sc[:tq, 0:nk],
                                                 axis=mybir.AxisListType.X,
                                                 negate=True)
                            pe = scrp.tile([128, 144], BF16, tag="pe")
                            sume = statp.tile([128, 1], F32, tag="sume")
                            nc.scalar.activation(pe[:tq, 0:nk], sc[:tq, 0:nk],
                                                 AF.Exp, bias=nmax[:tq],
                                                 scale=1.0, accum_out=sume[:tq])
                            rs = statp.tile([128, 1], F32, tag="rs")
                            nc.vector.reciprocal(rs[:tq], sume[:tq])
                            nc.vector.tensor_scalar_mul(pe[:tq, 0:nk], pe[:tq, 0:nk],
                                                        rs[:tq])
                            # transpose attn and multiply by shifted v
                            hs = slice(hh * 128, (hh + 1) * 128)
                            n0 = min(nk, 128)
                            py = pat2.tile([128, 128], F32, tag="py")
                            tb = scrp.tile([128, 128], BF16, tag="tb")
                            ptb = pat2.tile([128, 128], BF16, tag="ptb")
                            nc.tensor.transpose(ptb[:n0, :tq], pe[:tq, 0:n0],
                                                ident_sb[:tq, :tq])
                            nc.vector.tensor_copy(tb[:n0, :tq], ptb[:n0, :tq])
                            nc.tensor.matmul(
                                py[:, :tq], vt[:n0, tt, hs], tb[:n0, :tq],
                                start=True, stop=(nk <= 128))
                            if nk > 128:
                                ta = scrp.tile([16, 128], BF16, tag="ta")
                                pta = pat2.tile([16, 128], BF16, tag="pta")
                                nc.tensor.transpose(pta[:, :tq], pe[:tq, 128:nk],
                                                    ident_sb[:tq, :tq])
                                nc.vector.tensor_copy(ta[:, :tq], pta[:, :tq])
                                nc.tensor.matmul(
                                    py[:, :tq], vt[0:16, tt + 1, hs],
                                    ta[:, :tq], start=False, stop=True)
                            nc.vector.tensor_copy(y[:, hh, t0:t0 + tq], py[:, :tq])

                # ---- output projection + residual (attn_scale folded) ----
                with tc.tile_pool(name="po", bufs=4, space="PSUM") as pop:
                    for m in range(MC):
                        wts = []
                        for c in range(KC):
                            wt = wfp.tile([128, 128], F32R, tag="wf")
                            nc.sync.dma_start(wt[:], wo_d[l, c, m].bitcast(F32R))
                            wts.append(wt)
                        for t0, tn in NTS:
                            ts = slice(t0, t0 + tn)
                            po = pop.tile([128, tn], F32, tag="po")
                            for c in range(KC):
                                nc.tensor.matmul(po[:], wts[c][:], y[:, c, ts],
                                                 start=(c == 0), stop=(c == KC - 1))
                            nc.vector.tensor_add(h[:, m, ts], h[:, m, ts], po[:])

                # ---- ffn norm (fnw folded into w1/w3) ----
                xn = bigp.tile([128, KC, T], F32R, tag="xy", name=f"xn2_{l}")
                rms_norm_into(xn, eps_n)

                # ---- SwiGLU FFN (ffn_scale folded into w2) ----
                for sup in range(NSUP):
                    g = bigp.tile([128, KC, T], BF16, tag="g", name=f"g_{l}_{sup}")
                    with tc.tile_pool(name="pff", bufs=2, space="PSUM") as pff:
                        for fm in range(KC):
                            ff = sup * KC + fm
                            w1s, w3s = [], []
                            for c in range(KC):
                                wt1 = wfp.tile([128, 128], F32R, tag="wf")
                                nc.sync.dma_start(wt1[:],
                                                  w1_d[l, c, ff].bitcast(F32R))
                                w1s.append(wt1)
                                wt3 = wfp.tile([128, 128], F32R, tag="wf")
                                nc.sync.dma_start(wt3[:],
                                                  w3_d[l, c, ff].bitcast(F32R))
                                w3s.append(wt3)
                            for t0, tn in NTS:
                                ts = slice(t0, t0 + tn)
                                p1 = pff.tile([128, tn], F32, tag="p1")
                                p3 = pff.tile([128, tn], F32, tag="p3")
                                for c in range(KC):
                                    nc.tensor.matmul(p1[:], w1s[c][:], xn[:, c, ts],
                                                     start=(c == 0),
                                                     stop=(c == KC - 1))
                                for c in range(KC):
                                    nc.tensor.matmul(p3[:], w3s[c][:], xn[:, c, ts],
                                                     start=(c == 0),
                                                     stop=(c == KC - 1))
                                s1 = scrp.tile([128, tn], BF16, tag="s1")
                                if SIM_COMPAT:
                                    nc.scalar.activation(s1[:], p1[:], AF.Sigmoid)
                                    nc.vector.tensor_mul(s1[:], s1[:], p1[:])
                                else:
                                    nc.scalar.activation(s1[:], p1[:], AF.Silu)
                                nc.vector.tensor_mul(g[:, fm, ts], s1[:], p3[:])
                    with tc.tile_pool(name="pw2", bufs=4, space="PSUM") as pw2:
                        for m in range(MC):
                            wts = []
                            for c in range(KC):
                                wt = wbp.tile([128, 128], BF16, tag="wb")
                                nc.sync.dma_start(wt[:], w2_d[l, sup * KC + c, m])
                                wts.append(wt)
                            for t0, tn in NTS:
                                ts = slice(t0, t0 + tn)
                                p2 = pw2.tile([128, tn], F32, tag="p2")
                                for c in range(KC):
                                    nc.tensor.matmul(p2[:], wts[c][:], g[:, c, ts],
                                                     start=(c == 0),
                                                     stop=(c == KC - 1))
                                nc.vector.tensor_add(h[:, m, ts], h[:, m, ts],
                                                     p2[:])

            nc.sync.dma_start(out_t[:].rearrange("(c p) t -> p c t", p=128), h[:])

    nc.compile()
    return nc


def _prep_host(x, wq, wk, wv, wo, q_norm_w, k_norm_w, w1, w2, w3,
               attn_norm_w, ffn_norm_w, attn_scale, ffn_scale):
    f = np.float32
    x = np.asarray(x, f)
    anw = np.asarray(attn_norm_w, f)
    fnw = np.asarray(ffn_norm_w, f)
    asc = np.asarray(attn_scale, f)
    fsc = np.asarray(ffn_scale, f)

    wq_f = np.asarray(wq, f) * anw[:, :, None]
    wk_f = np.asarray(wk, f) * anw[:, :, None]
    wv_f = np.asarray(wv, f) * anw[:, :, None]
    wo_f = np.asarray(wo, f) * asc[:, None, :]
    w1_f = np.asarray(w1, f) * fnw[:, :, None]
    w3_f = np.asarray(w3, f) * fnw[:, :, None]
    w2_f = np.asarray(w2, f) * fsc[:, None, :]

    def tile4(w, mcount):
        # [L, K, M] -> [L, K/128, M/128, 128, 128]
        Ldim, K, M = w.shape
        return np.ascontiguousarray(
            w.reshape(Ldim, K // 128, 128, M // 128, 128).transpose(0, 1, 3, 2, 4))

    wq_t = tile4(wq_f, MC)
    wk_t = tile4(wk_f, MC)
    wv_t = tile4(wv_f, MC)
    wo_t = tile4(wo_f, MC)
    w1_t = tile4(w1_f, FC)
    w3_t = tile4(w3_f, FC)
    w2_t = tile4(w2_f, MC).astype(ml_dtypes.bfloat16)

    qnw_t = np.ascontiguousarray(
        (np.asarray(q_norm_w, f) / math.sqrt(HD)).reshape(L, KC, 128)
        .transpose(2, 0, 1))
    knw_t = np.ascontiguousarray(
        np.asarray(k_norm_w, f).reshape(L, KC, 128).transpose(2, 0, 1))

    # ALiBi band biases + masks: [128 queries, 144 keys], key j = t0 - 16 + c
    c_idx = np.arange(144)[None, :]
    p_idx = np.arange(128)[:, None]
    rel = (c_idx - p_idx - 16).astype(f)
    band_ok = (rel <= 0) & (rel >= -WINDOW)
    slopes = np.array([2.0 ** (-(hh + 1)) for hh in range(H)], f)
    bias_half = []
    for half in range(2):
        cut = 48 if half == 0 else 16   # mask keys j < 32 (pad) / j < 0
        slots = np.empty((H, 2, 128, 144), f)
        for hh in range(H):
            band = np.where(band_ok, slopes[hh] * rel, NEG)
            slots[hh, 0] = np.where(c_idx < cut, NEG, band)
            slots[hh, 1] = band
        arr = np.ascontiguousarray(
            slots.transpose(2, 0, 1, 3).reshape(128, H * 2, 144))
        bias_half.append(arr.astype(ml_dtypes.bfloat16))

    ones_t = np.ones((128, 128), ml_dtypes.bfloat16)
    ident_t = np.eye(128, dtype=ml_dtypes.bfloat16)

    shared = dict(wq_t=wq_t, wk_t=wk_t, wv_t=wv_t, wo_t=wo_t,
                  w1_t=w1_t, w3_t=w3_t, w2_t=w2_t,
                  qnw_t=qnw_t, knw_t=knw_t,
                  ones_t=ones_t, ident_t=ident_t)

    in_maps = []
    for core in range(8):
        b, half = core // 2, core % 2
        if half == 0:
            xc = np.concatenate(
                [np.zeros((D, HALO), f), x[b, 0:1024].T], axis=1)
        else:
            xc = np.ascontiguousarray(x[b, 1024 - HALO:2048].T)
        m = dict(shared)
        m["x_t"] = np.ascontiguousarray(xc)
        m["bias_t"] = bias_half[half]
        in_maps.append(m)
    return in_maps


def kernel(**inputs):
    if "nc" not in _CACHE:
        _CACHE["nc"] = _build_program()
    nc = _CACHE["nc"]

    in_maps = _prep_host(**inputs)
    res = run_bass_kernel_spmd(nc, in_maps, list(range(8)))

    out = np.empty((B, S, D), np.float32)
    for core in range(8):
        b, half = core // 2, core % 2
        out[b, half * 1024:(half + 1) * 1024, :] = \
            res.results[core]["out_t"][:, HALO:].T
    return out
